# revision 2
# baseline (speedup 1.0000x reference)
"""Trainium2 Bass kernel for a single attention head.

reference computation (fp32):
    q = query @ Wq + bq ; k = key @ Wk + bk ; v = value @ Wv + bv
    out = softmax((q @ k^T) / 8) @ v

Sharding: 8 cores, core c -> (batch b = c//2, query-half h = c%2).
Each core computes attention for its 2048 query rows against the full 4096
keys/values of its batch.

v2 design (bf16 pipeline, host-transposed inputs; rel err ~2e-3):
  - host supplies X^T slices in bf16 ([512, rows], c-major) so activations
    DMA straight into SBUF in the projection-ready layout: no PE transposes
    of X, no PSUM->SBUF staging copies, and half the HBM traffic of fp32.
  - projections on PE (bf16): lhsT = W [c-chunk, d], rhs = X^T chunk;
    bias folded into the mandatory PSUM->SBUF copy (DVE tensor_scalar_add).
    Qp^T [64, 2048] is duplicated to partitions 64:128; Kp^T is stored
    dual-half (even j-chunks on partitions 0:64, odd on 64:128) by issuing
    separate even/odd matmuls whose outputs land at partition offsets 0/64
    via tile_position, so no partition-shift DMA is needed.
  - V is projected then PE-transposed to natural [rows, 66]; col 64 is
    all-ones (host pads Wv/bv) so the PV matmul also produces the softmax
    denominator; col 65 is zero padding.
  - scores^T tiles: lhsT = Kp^T[half, j-chunk] [64,128], rhs = Qp^T
    [64, 1024-i-slice] -> S^T [128 j, i] in PSUM; the two K=64 matmuls of a
    j-chunk pair occupy PE row-groups 0:64 / 64:128 (tile_position row
    tiling); exp fused with the 1/8 scale on ScalarE, output bf16 (no
    max-subtraction: |scores/8| <= ~3 so fp32 exp is safe).
  - PV: lhsT = v[j-chunk] [128, 66] bf16, rhs = P^T [128, i] bf16,
    accumulated over j in PSUM -> out^T [66, i] (row 64 = denominator).
  - epilogue: PE-transpose out^T, reciprocal + scale on DVE, DMA out.
  - attention for the first i-half is interleaved with the k/v prework;
    q blocks 2,3 (cols 1024:2048, only needed for i-half 1) are deferred
    into the k/v loop.
"""

import sys

if "/opt/trn_rl_repo" not in sys.path:
    sys.path.insert(0, "/opt/trn_rl_repo")

from contextlib import ExitStack

import numpy as np
import ml_dtypes

import concourse.bass as bass
import concourse.tile as tile
from concourse import bacc, mybir
from concourse.bass_utils import run_bass_kernel_spmd
from concourse.masks import make_identity

F32 = mybir.dt.float32
BF16 = mybir.dt.bfloat16
NP_BF16 = ml_dtypes.bfloat16

B, S, C, D = 4, 4096, 512, 64
D2 = D + 2          # v padded with [ones, zeros] cols
N_CORES = 8
SQ = S // 2          # query rows per core
NJ = S // 128        # 32 key chunks of 128 rows
NP_ = NJ // 2        # 16 chunk pairs
IH = SQ // 2         # 1024: i-half processed per PSUM residency
EXP = mybir.ActivationFunctionType.Exp

_CACHE = {}


def _emit(nc, tc, aps):
    qT_d, kT_d, vT_d, wq_d, wk_d, wvp_d, bq_d, bk2_d, bvp_d, out_d = aps

    ctx = ExitStack()
    const = ctx.enter_context(tc.tile_pool(name="const", bufs=1))
    persist = ctx.enter_context(tc.tile_pool(name="persist", bufs=1))
    stage_p = ctx.enter_context(tc.tile_pool(name="stage", bufs=3))
    vt_p = ctx.enter_context(tc.tile_pool(name="vt", bufs=2))
    pt_p = ctx.enter_context(tc.tile_pool(name="pt", bufs=4))
    ep_p = ctx.enter_context(tc.tile_pool(name="ep", bufs=2))
    small_p = ctx.enter_context(tc.tile_pool(name="small", bufs=4))
    out_p = ctx.enter_context(tc.tile_pool(name="outp", bufs=2))
    # PSUM budget (8 banks): scratch 2x1 + st 2x2 + po 1x2 = 8
    pp_ps = ctx.enter_context(tc.tile_pool(name="ppps", bufs=2, space="PSUM"))
    st_ps = ctx.enter_context(tc.tile_pool(name="stps", bufs=2, space="PSUM"))
    po_ps = ctx.enter_context(tc.tile_pool(name="pops", bufs=1, space="PSUM"))

    ident32 = const.tile([128, 128], F32)
    make_identity(nc, ident32[:])
    ident = const.tile([128, 128], BF16)
    nc.vector.tensor_copy(ident[:], ident32[:])

    wq_sb = const.tile([128, 4, D], BF16)
    nc.sync.dma_start(wq_sb[:], wq_d.rearrange("(cc p) d -> p cc d", p=128))
    wk_sb = const.tile([128, 4, D], BF16)
    nc.sync.dma_start(wk_sb[:], wk_d.rearrange("(cc p) d -> p cc d", p=128))
    wvp_sb = const.tile([128, 4, D2], BF16)
    nc.sync.dma_start(wvp_sb[:], wvp_d.rearrange("(cc p) d -> p cc d", p=128))
    bq_sb = const.tile([D, 1], F32)
    nc.sync.dma_start(bq_sb[:], bq_d[:])
    bk2_sb = const.tile([128, 1], F32)       # k bias duplicated on both halves
    nc.sync.dma_start(bk2_sb[:], bk2_d[:])
    bvp_sb = const.tile([D2, 1], F32)
    nc.sync.dma_start(bvp_sb[:], bvp_d[:])

    qpt = persist.tile([128, SQ], BF16)      # Qp^T duplicated on both halves
    kpt = persist.tile([128, S // 2], BF16)  # Kp^T dual-half (even|odd chunks)
    v_sb = persist.tile([128, NJ, D2], BF16)  # v natural + ones col

    def load_block(x_d, g):
        """DMA 512 c x 512 rows of a host-transposed activation into SBUF."""
        stg = stage_p.tile([128, 4, 512], BF16, tag="stage")
        nc.gpsimd.dma_start(
            stg[:],
            x_d[:, g * 512 : (g + 1) * 512].rearrange("(cc p) r -> p cc r", p=128),
        )
        return stg

    def proj_q(g):
        stg = load_block(qT_d, g)
        pp = pp_ps.tile([128, 512], F32, tag="pp")
        for cc in range(4):
            nc.tensor.matmul(
                pp[:D, :], wq_sb[:, cc, :], stg[:, cc, :],
                start=(cc == 0), stop=(cc == 3),
            )
        sl = slice(g * 512, (g + 1) * 512)
        nc.vector.tensor_scalar_add(qpt[:D, sl], pp[:D, :], bq_sb[:])
        nc.sync.dma_start(qpt[D:, sl], qpt[:D, sl])

    def proj_k(g):
        # block g covers j-chunks 4g..4g+3; even chunks project to output
        # partitions 0:64, odd to 64:128 (tile_position col offset), so the
        # bias-add writes kpt's dual-half layout directly.
        stg = load_block(kT_d, g)
        pp = pp_ps.tile([128, 512], F32, tag="pp")
        for half in range(2):
            for cc in range(4):
                rhs = stg[:, cc, :].rearrange("p (c n) -> p c n", n=128)[:, half::2, :]
                nc.tensor.matmul(
                    pp[half * D : (half + 1) * D, :256],
                    wk_sb[:, cc, :],
                    rhs,
                    start=(cc == 0), stop=(cc == 3),
                    tile_position=(0, half * D),
                )
        sl = slice(g * 256, (g + 1) * 256)
        nc.vector.tensor_scalar_add(kpt[:D, sl], pp[:D, :256], bk2_sb[:D, :])
        nc.vector.tensor_scalar_add(kpt[D:, sl], pp[D:, :256], bk2_sb[D:, :])

    def proj_v(g):
        stg = load_block(vT_d, g)
        pp = pp_ps.tile([128, 512], F32, tag="pp")
        for cc in range(4):
            nc.tensor.matmul(
                pp[:D2, :], wvp_sb[:, cc, :], stg[:, cc, :],
                start=(cc == 0), stop=(cc == 3),
            )
        vt = vt_p.tile([D2, 512], BF16, tag="vt")
        nc.vector.tensor_scalar_add(vt[:], pp[:D2, :], bvp_sb[:])
        for r in range(4):
            vnp = pp_ps.tile([128, D2], BF16, tag="pp")
            nc.tensor.transpose(
                vnp[:], vt[:, r * 128 : (r + 1) * 128], ident[:D2, :D2]
            )
            nc.vector.tensor_copy(v_sb[:, g * 4 + r, :], vnp[:])

    def attention(p, ih, po, first, last):
        # chunk pair p = chunks (2p, 2p+1): even on kpt rows 0:64, odd 64:128
        sts = []
        for half in range(2):
            st = st_ps.tile([128, IH], F32, tag="st")
            for n in range(IH // 512):
                nc.tensor.matmul(
                    st[:, n * 512 : (n + 1) * 512],
                    kpt[half * D : (half + 1) * D, p * 128 : (p + 1) * 128],
                    qpt[half * D : (half + 1) * D,
                        ih * IH + n * 512 : ih * IH + (n + 1) * 512],
                    tile_position=(half * D, 0),
                )
            pt = pt_p.tile([128, IH], BF16, tag="pt")
            nc.scalar.activation(pt[:], st[:], EXP, scale=0.125)
            sts.append(pt)
        for half in range(2):
            for n in range(IH // 512):
                nc.tensor.matmul(
                    po[:, n * 512 : (n + 1) * 512],
                    v_sb[:, 2 * p + half, :],
                    sts[half][:, n * 512 : (n + 1) * 512],
                    start=(first and half == 0), stop=(last and half == 1),
                )

    def epilogue(ih, po):
        ot = ep_p.tile([D2, IH], BF16, tag="ot")
        nc.vector.tensor_copy(ot[:], po[:])
        osb = out_p.tile([128, IH // 128, D], F32, tag="osb")
        for t in range(IH // 128):
            onat = pp_ps.tile([128, D2], BF16, tag="pp")
            nc.tensor.transpose(
                onat[:], ot[:, t * 128 : (t + 1) * 128], ident[:D2, :D2]
            )
            rs = small_p.tile([128, 1], F32, tag="rs")
            nc.vector.reciprocal(rs[:], onat[:, D : D + 1])
            nc.vector.tensor_scalar_mul(osb[:, t, :], onat[:, :D], rs[:])
        nc.sync.dma_start(
            out_d[ih * IH : (ih + 1) * IH, :].rearrange("(t p) d -> p t d", p=128),
            osb[:],
        )

    # q cols 0:1024 feed i-half 0; blocks 2,3 deferred into the k/v loop
    for g in range(2):
        proj_q(g)

    po0 = po_ps.tile([D2, IH], F32, tag="po")
    for g in range(8):
        proj_k(g)
        proj_v(g)
        if g < 2:
            proj_q(2 + g)
        for p in (2 * g, 2 * g + 1):
            attention(p, 0, po0, first=(p == 0), last=(p == NP_ - 1))
    epilogue(0, po0)

    po1 = po_ps.tile([D2, IH], F32, tag="po")
    for p in range(NP_):
        attention(p, 1, po1, first=(p == 0), last=(p == NP_ - 1))
    epilogue(1, po1)
    ctx.close()


def _build(reps=1):
    nc = bacc.Bacc("TRN2", target_bir_lowering=False, debug=False, num_devices=N_CORES)
    aps = (
        nc.dram_tensor("qT", [C, SQ], BF16, kind="ExternalInput").ap(),
        nc.dram_tensor("kT", [C, S], BF16, kind="ExternalInput").ap(),
        nc.dram_tensor("vT", [C, S], BF16, kind="ExternalInput").ap(),
        nc.dram_tensor("wq", [C, D], BF16, kind="ExternalInput").ap(),
        nc.dram_tensor("wk", [C, D], BF16, kind="ExternalInput").ap(),
        nc.dram_tensor("wvp", [C, D2], BF16, kind="ExternalInput").ap(),
        nc.dram_tensor("bq", [D, 1], F32, kind="ExternalInput").ap(),
        nc.dram_tensor("bk2", [128, 1], F32, kind="ExternalInput").ap(),
        nc.dram_tensor("bvp", [D2, 1], F32, kind="ExternalInput").ap(),
        nc.dram_tensor("out", [SQ, D], F32, kind="ExternalOutput").ap(),
    )
    with tile.TileContext(nc) as tc:
        for _ in range(reps):
            _emit(nc, tc, aps)
    nc.compile()
    return nc


def get_nc():
    if "nc" not in _CACHE:
        _CACHE["nc"] = _build()
    return _CACHE["nc"]


def make_in_maps(query, key_, value, Wq, bq, Wk, bk, Wv, bv):
    query, key_, value, Wq, bq, Wk, bk, Wv, bv = (
        np.asarray(a, dtype=np.float32)
        for a in (query, key_, value, Wq, bq, Wk, bk, Wv, bv)
    )
    wvp = np.concatenate([Wv, np.zeros((C, 2), np.float32)], axis=1)
    bvp = np.concatenate([bv, np.asarray([1.0, 0.0], np.float32)])[:, None]
    bk2 = np.concatenate([bk, bk])[:, None]
    shared = {
        "wq": np.ascontiguousarray(Wq.astype(NP_BF16)),
        "wk": np.ascontiguousarray(Wk.astype(NP_BF16)),
        "wvp": np.ascontiguousarray(wvp.astype(NP_BF16)),
        "bq": np.ascontiguousarray(bq[:, None]),
        "bk2": np.ascontiguousarray(bk2),
        "bvp": np.ascontiguousarray(bvp),
    }
    kT = [np.ascontiguousarray(key_[b].T.astype(NP_BF16)) for b in range(B)]
    vT = [np.ascontiguousarray(value[b].T.astype(NP_BF16)) for b in range(B)]
    in_maps = []
    for c in range(N_CORES):
        b, h = divmod(c, 2)
        in_maps.append(
            {
                "qT": np.ascontiguousarray(
                    query[b, h * SQ : (h + 1) * SQ, :].T.astype(NP_BF16)
                ),
                "kT": kT[b],
                "vT": vT[b],
                **shared,
            }
        )
    return in_maps


def assemble(results):
    out = np.empty((B, S, D), np.float32)
    for c in range(N_CORES):
        b, h = divmod(c, 2)
        out[b, h * SQ : (h + 1) * SQ, :] = results[c]["out"]
    return out


def kernel(query=None, key_=None, value=None, Wq=None, bq=None, Wk=None,
           bk=None, Wv=None, bv=None, key=None, **_):
    if key_ is None:
        key_ = key          # spec names this input "key"; reference uses "key_"
    nc = get_nc()
    in_maps = make_in_maps(query, key_, value, Wq, bq, Wk, bk, Wv, bv)
    res = run_bass_kernel_spmd(nc, in_maps, list(range(N_CORES)))
    return assemble(res.results)
